# revision 20
# baseline (speedup 1.0000x reference)
"""Trainium2 Bass kernel for nn_Encoder (GRU over I timesteps).

Problem shapes (hardcoded): B=64, N=16, I=128, H=256, V=128.
Sharding: data-parallel over B across 8 cores -> 128 sequences/core,
which exactly fills the 128 SBUF partitions (S-layout: partition = sequence).

Per-core recurrence (per timestep t), organized to minimize the serial
dependency chain:
  G_r  = onehot(x_t) @ Wr' + h @ Whh_r        (PE, PSUM accum; float32r)
  G_z  = onehot(x_t) @ Wz' + h @ Whh_z
  Xn   = onehot(x_t) @ Wn'
  Hn   = b_hh_n + h @ Whh_n                    (bias via K=1 ones-row matmul)
  r    = sigmoid(G_r)           z  = sigmoid(G_z)       zc = sigmoid(-G_z) = 1-z
  n    = tanh(Xn + r*Hn)
  h'   = zc*n + z*h   computed TRANSPOSED:  h'T = zcT*nT + zT*hT
where nT/zT/zcT come from PE transposes. The state lives only in T-layout
(hT, SBUF), which is exactly the lhsT the next step's matmuls need -- no
PSUM->SBUF copy sits on the recurrence chain. History is emitted in T-layout
and untransposed on the host (host work is free w.r.t. HW kernel time).

The matmul dtype is float32r (fp32 bits, full-rate PE streaming at N>=256;
plain fp32 matmuls run at 1/4 rate). zT is consumed straight from PSUM; only
zcT needs an SBUF copy (DVE tensor ops allow at most one PSUM operand).
"""

import os
import numpy as np

import concourse.bass as bass
import concourse.bacc as bacc
import concourse.tile as tile
from concourse import mybir
from concourse.bass_utils import run_bass_kernel_spmd
from contextlib import ExitStack

B, N, I, H, V = 64, 16, 128, 256, 128
NCORES = 8
BPC = B // NCORES          # batch rows per core
S = BPC * N                # sequences per core = 128
H3 = 3 * H
F32 = mybir.dt.float32
F32R = mybir.dt.float32r
AF = mybir.ActivationFunctionType
ALU = mybir.AluOpType


def _build_nc(steps=I):
    nc = bacc.Bacc(None, target_bir_lowering=False)
    # weight layout (columns): [r 0:256 | z 256:512 | n 512:768]
    oh_d = nc.declare_dram_parameter("oh", [steps, V, S], F32R, False)
    wih_d = nc.declare_dram_parameter("wih", [V, H3], F32R, False)
    whh_d = nc.declare_dram_parameter("whh", [2, 128, H3], F32R, False)
    bhn_d = nc.declare_dram_parameter("bhn", [1, H], F32R, False)
    idn_d = nc.declare_dram_parameter("idn", [128, 128], F32, False)
    ones_d = nc.declare_dram_parameter("ones", [1, S], F32R, False)
    # history in T-layout: [t, partition p, chunk c, seq s] -> h-unit = c*128+p
    hist_d = nc.declare_dram_parameter("hist", [steps, 128, 2, S], F32R, True)

    with ExitStack() as ctx:
        tc = ctx.enter_context(tile.TileContext(nc))
        consts = ctx.enter_context(tc.tile_pool(name="consts", bufs=1))
        ohp = ctx.enter_context(tc.tile_pool(name="ohp", bufs=4))
        state = ctx.enter_context(tc.tile_pool(name="state", bufs=3))
        work = ctx.enter_context(tc.tile_pool(name="work", bufs=3))
        # PSUM pools, 7 banks. Banks are engine-partitioned: ACT only ever
        # reads pgrz/pzct banks, DVE only reads phx/pnt/pzt banks -- a PSUM
        # bank read by two engines makes Tile serialize them (same-bank
        # ScalarE+VectorE access hazard), which would put ~600ns of false
        # waits on the recurrence chain.
        pgr = ctx.enter_context(tc.tile_pool(name="pgr", bufs=2, space="PSUM"))
        pgz = ctx.enter_context(tc.tile_pool(name="pgz", bufs=2, space="PSUM"))
        phx = ctx.enter_context(tc.tile_pool(name="phx", bufs=2, space="PSUM"))
        pat = ctx.enter_context(tc.tile_pool(name="pat", bufs=1, space="PSUM"))
        pzt = ctx.enter_context(tc.tile_pool(name="pzt", bufs=1, space="PSUM"))

        wih = consts.tile([V, H3], F32R)
        nc.sync.dma_start(wih, wih_d[:])
        whh = consts.tile([128, 2, H3], F32R)
        nc.sync.dma_start(whh[:, 0, :], whh_d[0])
        nc.sync.dma_start(whh[:, 1, :], whh_d[1])
        bhn = consts.tile([1, H], F32R)
        nc.sync.dma_start(bhn, bhn_d[:])
        idn = consts.tile([128, 128], F32)
        nc.sync.dma_start(idn, idn_d[:])
        ones = consts.tile([1, S], F32R)
        nc.sync.dma_start(ones, ones_d[:])

        hT = None  # state: (128, 2, 128) F32R (rounded by the producing DVE op)

        def emit_xside(t):
            """Allocate step-t tiles and emit its x-side matmuls (bias, xn,
            g_r's opening matmul). Called one step AHEAD so these fill PE
            idle slots during the previous step's elementwise tail instead of
            queueing behind its transposes (PE is a strict FIFO).

            PSUM rule: a start=True matmul clears has_written for its whole
            bank, so all start-groups for a bank precede any accumulation
            into it. g_z's opening matmul is the exception -- it must wait
            until g_r's accumulation is complete, so it is emitted with the
            h-side matmuls."""
            oh = ohp.tile([V, S], F32R, tag="oh")
            nc.sync.dma_start(oh, oh_d[t])
            g_r = pgr.tile([128, 256], F32, tag="gr")     # ACT-read bank
            g_z = pgz.tile([128, 256], F32, tag="gz")     # ACT-read bank
            hx = phx.tile([128, 512], F32, tag="hx")      # [hn|xn]  DVE-read
            first = (t == 0)
            nc.tensor.matmul(hx[:, 256:512], oh[:], wih[:, 512:768],
                             start=True, stop=True)
            nc.tensor.matmul(hx[:, 0:256], ones[:], bhn[:],
                             start=True, stop=first)
            nc.tensor.matmul(g_r[:], oh[:], wih[:, 0:256],
                             start=True, stop=first)
            nc.tensor.matmul(g_z[:], oh[:], wih[:, 256:512],
                             start=True, stop=first)
            return oh, g_r, g_z, hx

        cur = emit_xside(0)

        for t in range(steps):
            first = (t == 0)
            oh, g_r, g_z, hx = cur
            hn = hx[:, 0:256]
            xn = hx[:, 256:512]

            if not first:
                hTr = hT[:]
                # r-gate first: it heads the serial chain
                nc.tensor.matmul(g_r[:], hTr[:, 0, :], whh[:, 0, 0:256],
                                 start=False, stop=False)
                nc.tensor.matmul(g_r[:], hTr[:, 1, :], whh[:, 1, 0:256],
                                 start=False, stop=True)
                nc.tensor.matmul(g_z[:], hTr[:, 0, :], whh[:, 0, 256:512],
                                 start=False, stop=False)
                nc.tensor.matmul(g_z[:], hTr[:, 1, :], whh[:, 1, 256:512],
                                 start=False, stop=True)
                nc.tensor.matmul(hn, hTr[:, 0, :], whh[:, 0, 512:768],
                                 start=False, stop=False)
                nc.tensor.matmul(hn, hTr[:, 1, :], whh[:, 1, 512:768],
                                 start=False, stop=True)
            if t + 1 < steps:
                cur = emit_xside(t + 1)

            r_sb = work.tile([128, 256], F32, tag="r")
            nc.scalar.activation(r_sb[:], g_r[:], AF.Sigmoid)
            # z second: it feeds the z-transposes -> bT, which must clear the
            # DVE before a arrives
            z_sb = work.tile([128, 256], F32, tag="z")
            nc.scalar.activation(z_sb[:], g_z[:], AF.Sigmoid)
            zc_sb = work.tile([128, 256], F32, tag="zc")
            nc.scalar.activation(zc_sb[:], g_z[:], AF.Sigmoid, scale=-1.0)

            # n = tanh(xn + r*hn)
            v = work.tile([128, 256], F32, tag="v")
            nc.vector.tensor_tensor(out=v[:], in0=r_sb[:], in1=hn[:], op=ALU.mult)
            nc.vector.tensor_tensor(out=v[:], in0=v[:], in1=xn, op=ALU.add)
            n_sb = work.tile([128, 256], F32, tag="n")
            nc.scalar.activation(n_sb[:], v[:], AF.Tanh)

            # z transposed for bT (off-chain; own bank, DVE-read only)
            zt = pzt.tile([128, 2, 128], F32, tag="zt")
            nc.tensor.transpose(zt[:, 0, :], z_sb[:, 0:128], idn[:])
            nc.tensor.transpose(zt[:, 1, :], z_sb[:, 128:256], idn[:])

            hT_new = state.tile([128, 2, 128], F32R, tag="hT")
            if not first:
                # bT = zT*hT: ready mid-step, must precede `a` in the DVE FIFO
                bT = work.tile([128, 2, 128], F32, tag="bT")
                nc.vector.tensor_tensor(out=bT[:], in0=hT[:],
                                        in1=zt[:], op=ALU.mult)

            # chain tail: a = zc*n in S-layout (SBUF ops, right after tanh),
            # THEN transpose a -- no PSUM->SBUF copy sits on the chain.
            a_sb = work.tile([128, 256], F32, tag="a")
            nc.vector.tensor_tensor(out=a_sb[:], in0=zc_sb[:], in1=n_sb[:],
                                    op=ALU.mult)
            at = pat.tile([128, 2, 128], F32, tag="at")
            nc.tensor.transpose(at[:, 0, :], a_sb[:, 0:128], idn[:])
            nc.tensor.transpose(at[:, 1, :], a_sb[:, 128:256], idn[:])

            if first:
                nc.vector.tensor_copy(hT_new[:], at[:])
            else:
                nc.vector.tensor_tensor(out=hT_new[:], in0=at[:],
                                        in1=bT[:], op=ALU.add)

            nc.sync.dma_start(hist_d[t], hT_new[:])
            hT = hT_new
    nc.compile()
    return nc


_CACHE = {}
LAST_EXEC_NS = None


def _get_nc(steps=I):
    key = ("nc", steps)
    if key not in _CACHE:
        _CACHE[key] = _build_nc(steps)
    return _CACHE[key]


def kernel(input, input_lengths, W_ih, b_ih, W_hh, b_hh, steps=I):
    global LAST_EXEC_NS
    input = np.asarray(input)
    input_lengths = np.asarray(input_lengths)
    W_ih = np.asarray(W_ih, dtype=np.float32)
    b_ih = np.asarray(b_ih, dtype=np.float32)
    W_hh = np.asarray(W_hh, dtype=np.float32)
    b_hh = np.asarray(b_hh, dtype=np.float32)

    wih = (W_ih + b_ih[None, :]).astype(np.float32)
    wih[:, 0:2 * H] += b_hh[None, 0:2 * H]
    whh = np.ascontiguousarray(
        np.stack([W_hh[0:128], W_hh[128:256]]), dtype=np.float32)
    bhn = np.ascontiguousarray(b_hh[2 * H:].reshape(1, H), dtype=np.float32)
    idn = np.eye(128, dtype=np.float32)

    in_maps = []
    for c in range(NCORES):
        ids = input[c * BPC:(c + 1) * BPC].reshape(S, I)[:, :steps]
        oh = np.zeros((steps, V, S), np.float32)
        oh[np.arange(steps)[None, :].repeat(S, 0).T.reshape(-1),
           ids.T.reshape(-1),
           np.arange(S)[None, :].repeat(steps, 0).reshape(-1)] = 1.0
        in_maps.append({"oh": oh, "wih": wih, "whh": whh, "bhn": bhn,
                        "idn": idn, "ones": np.ones((1, S), np.float32)})

    trace = os.environ.get("GRU_TRACE") == "1"
    if not trace:
        # NTFF tracing needs an axon hook that is absent in this
        # environment; make sure a stray BASS_TRACE can't select it.
        os.environ["BASS_NEVER_TRACE"] = "1"
    res = run_bass_kernel_spmd(_get_nc(steps), in_maps, list(range(NCORES)),
                               trace=trace)
    LAST_EXEC_NS = res.exec_time_ns

    history = np.empty((B, N, steps, H), np.float32)
    for c in range(NCORES):
        hist = res.results[c]["hist"]          # (steps, 128, 2, S)
        # h-unit index = c*128 + p  ->  (S, steps, H)
        hc = hist.transpose(3, 0, 2, 1).reshape(S, steps, H)
        history[c * BPC:(c + 1) * BPC] = hc.reshape(BPC, N, steps, H)
    if steps != I:
        return history, None
    idx = (input_lengths - 1)[:, :, None, None]
    last = np.take_along_axis(history, idx, axis=2)[:, :, 0, :]
    mean = (last.sum(axis=1) / np.float32(N)).astype(np.float32)
    return history, mean
